# revision 34
# baseline (speedup 1.0000x reference)
"""Trainium2 Bass kernel for nn_CodaAttention (GQA attention with depth-KV
prefix, QK-norm, RoPE, XSA value-projection subtraction).

Sharding: tensor-parallel over heads across 8 cores. Core c owns q-heads
{2c, 2c+1} and kv-head c//2. Pipeline over 512-token chunks:
projections(+RoPE/QK-norm) for chunk n, attention for query group n,
per-chunk AllGather of y, wo matmuls two chunks later.

v2 notes (vs v1):
- Weights / x / wo-rhs are host-packed so each SBUF load is one large
  contiguous DMA (6 weight DMAs instead of 96; 2 x-DMAs per chunk).
- Scalar engine runs ONLY Ln/Exp (one activation table set) - squares,
  copies and norm muls moved to DVE/GpSimd, killing ACT_TABLE_LOAD thrash.
- V transpose via PE (tensor.transpose) instead of a DRAM round-trip
  DMA-transpose.
- Attention uses the 64-shifted key-tile grid with EXACT causal masks:
  tile kt covers keys [128kt-64, 128kt+64) (tile 0 starts with the 64
  depth keys), diagonal tiles are column-narrowed, and the last 64 keys
  of each group go through a 64-partition augmented tile.
- Elementwise ops stay bf16-in-SBUF where possible (DVE 4x mode).
"""
import os
import sys

sys.path.insert(0, "/opt/trn_rl_repo")

import numpy as np

import concourse.bass as bass
import concourse.mybir as mybir
import concourse.tile as tile
from concourse import bacc

DT = mybir.dt
F32, BF16 = DT.float32, DT.bfloat16
AF = mybir.ActivationFunctionType
ALU = mybir.AluOpType

KDBG = int(os.environ.get("KDBG", "0"))

B, T, DIM = 2, 2048, 2048
H, KVH, HD = 16, 4, 128
TD = 64
NCORES = 8
HPC = H // NCORES            # q heads per core = 2
TOK = B * T                  # 4096 flattened tokens
NKD = DIM // 128             # 16 contraction tiles
NCH = T // 512               # 4 query groups (512-token chunks) per batch
SCALE = 1.0 / np.sqrt(HD)


def _build():
    nc = bacc.Bacc("TRN2", target_bir_lowering=False, debug=False,
                   num_devices=NCORES)

    def inp(name, shape, dt=F32):
        return nc.dram_tensor(name, list(shape), dt,
                              kind="ExternalInput").ap()

    # host-packed inputs (see _shard_inputs)
    x_bf = inp("x_bf", (B * NCH, 128, NKD * 512), BF16)
    wq_c = inp("wq_c", (HPC, 128, NKD * 128), BF16)
    wk_c = inp("wk_c", (128, NKD * 128), BF16)
    wv_c = inp("wv_c", (128, NKD * 128), BF16)
    wo_c = inp("wo_c", (HPC, 128, NKD * 128), BF16)
    vb_ch = inp("vb_ch", (B * NCH, HD, 512), BF16)   # value_bias^T chunks
    dkT_c = inp("dkT_c", (B, HD, TD), BF16)          # transposed depth_k
    dv_c = inp("dv_c", (B, TD, HD), BF16)
    cosT = inp("cosT", (HD, T), BF16)                # pair-duplicated cos
    sinT = inp("sinT", (HD, T), BF16)                # pair-dup sign-folded sin
    qs_c = inp("qs_c", (128, HPC))                   # q_scale per local head
    ks_c = inp("ks_c", (128, 1))                     # k_scale, bcast

    outT = nc.dram_tensor("outT", [HPC * HD, TOK], BF16,
                          kind="ExternalOutput").ap()
    dbg = nc.dram_tensor("dbg", [128, 8192], BF16,
                         kind="ExternalOutput").ap()

    y_m7 = [nc.dram_tensor(f"y_m7{s}", [128, HPC, 256], BF16).ap()
            for s in range(2)]
    y_a7 = [nc.dram_tensor(f"y_a7{s}", [NCORES, 128, HPC, 256], BF16,
                           addr_space="Shared").ap() for s in range(2)]
    dum_i = nc.dram_tensor("dum_i", [128], BF16).ap()
    dum_o = nc.dram_tensor("dum_o", [NCORES * 128], BF16,
                           addr_space="Shared").ap()

    # DRAM scratch: per-chunk y (AG input must be internal DRAM)
    y_mine = [nc.dram_tensor(f"y_mine{c}", [128, HPC, 512], BF16).ap()
              for c in range(B * NCH)]
    y_all = [nc.dram_tensor(f"y_all{c}", [NCORES, 128, HPC, 512], BF16,
                            addr_space="Shared").ap() for c in range(B * NCH)]

    with tile.TileContext(nc) as tc:
        _emit(nc, tc, locals())
    nc.compile()
    return nc


def _emit(nc, tc, v):
    x_bf, wq_c, wk_c, wv_c, wo_c = (v["x_bf"], v["wq_c"], v["wk_c"],
                                    v["wv_c"], v["wo_c"])
    vb_ch, dkT_c, dv_c, cosT, sinT = (v["vb_ch"], v["dkT_c"], v["dv_c"],
                                      v["cosT"], v["sinT"])
    qs_c, ks_c, outT = v["qs_c"], v["ks_c"], v["outT"]
    dbg = v["dbg"]
    y_mine, y_all = v["y_mine"], v["y_all"]
    y_m7, y_a7 = v["y_m7"], v["y_a7"]
    dum_i, dum_o = v["dum_i"], v["dum_o"]

    # ---------------- pools ----------------
    const = tc.alloc_tile_pool(name="const", bufs=1)
    wpool = tc.alloc_tile_pool(name="wpool", bufs=1)
    big = tc.alloc_tile_pool(name="big", bufs=1)
    xp = tc.alloc_tile_pool(name="xp", bufs=2)
    rp = tc.alloc_tile_pool(name="rope", bufs=2)
    vbp = tc.alloc_tile_pool(name="vb", bufs=2)
    ap_sb = tc.alloc_tile_pool(name="attn_sb", bufs=2)
    wop = tc.alloc_tile_pool(name="wo_rhs", bufs=2)
    wos = tc.alloc_tile_pool(name="wo_sb", bufs=2)
    # PSUM: 8 banks total: pp 2 + pl 2 + pt 2 + py 1 + pz 1
    pp = tc.alloc_tile_pool(name="pp", bufs=2, space="PSUM")  # proj + wo
    pl = tc.alloc_tile_pool(name="pl", bufs=2, space="PSUM")  # logits + transp
    pt = tc.alloc_tile_pool(name="pt", bufs=2, space="PSUM")  # ss/vns/dot
    py = tc.alloc_tile_pool(name="py", bufs=1, space="PSUM")
    pz = tc.alloc_tile_pool(name="pz", bufs=1, space="PSUM")

    # ---------------- constants ----------------
    cos_sb = const.tile([HD, T], BF16, tag="cos")
    sin_sb = const.tile([HD, T], BF16, tag="sin")
    nc.sync.dma_start(out=cos_sb[:, :], in_=cosT[:, :])
    nc.sync.dma_start(out=sin_sb[:, :], in_=sinT[:, :])
    qs_sb = const.tile([128, HPC], F32, tag="qs")
    ks_sb = const.tile([128, 1], F32, tag="ks")
    nc.scalar.dma_start(out=qs_sb[:, :], in_=qs_c[:, :])
    nc.scalar.dma_start(out=ks_sb[:, :], in_=ks_c[:, :])
    ones_bf = const.tile([128, 128], BF16, tag="ones")
    nc.gpsimd.memset(ones_bf[:, :], 1.0)
    eps_sb = const.tile([128, 1], F32, tag="eps")
    nc.gpsimd.memset(eps_sb[:, :], 1e-12)
    # ones scaled by 1/qs^2 (per local q head) and 1/ks^2: folding the
    # norm scale into the sum-of-squares matmul makes the rsqrt
    # exp(-0.5*ln(ss/qs^2)) = qs/sqrt(ss) bias-free, so ONE wide Exp
    # covers all three chains (forces Ln/Ln/Ln/Exp table batching).
    sqq = const.tile([128, HPC], F32, tag="sqq")
    nc.vector.tensor_mul(sqq[:, :], qs_sb[:, :], qs_sb[:, :])
    rqq = const.tile([128, HPC], F32, tag="rqq")
    nc.vector.reciprocal(out=rqq[:, :], in_=sqq[:, :])
    sqk = const.tile([128, 1], F32, tag="sqk")
    nc.vector.tensor_mul(sqk[:, :], ks_sb[:, :], ks_sb[:, :])
    rqk = const.tile([128, 1], F32, tag="rqk")
    nc.vector.reciprocal(out=rqk[:, :], in_=sqk[:, :])
    ones_q = []
    for h in range(HPC):
        t = const.tile([128, 128], BF16, tag=f"onq{h}", name=f"onq{h}")
        nc.scalar.activation(t[:, :], ones_bf[:, :], AF.Copy,
                             scale=rqq[:, h:h + 1])
        ones_q.append(t)
    ones_k = const.tile([128, 128], BF16, tag="onk")
    nc.scalar.activation(ones_k[:, :], ones_bf[:, :], AF.Copy,
                         scale=rqk[:, 0:1])

    # identity for PE transpose
    ident = const.tile([128, 128], BF16, tag="ident")
    nc.gpsimd.memset(ident[:, :], 1.0)
    nc.gpsimd.affine_select(out=ident[:, :], in_=ident[:, :],
                            compare_op=ALU.is_equal, fill=0.0,
                            base=0, channel_multiplier=-1,
                            pattern=[[1, 128]])

    # causal masks (keep where c >= p + d), 0/1 bf16
    def affmask(tag, d):
        m = const.tile([128, 512], BF16, tag=tag, name=tag)
        nc.gpsimd.memset(m[:, :], 1.0)
        nc.gpsimd.affine_select(out=m[:, :], in_=m[:, :],
                                compare_op=ALU.is_ge, fill=0.0,
                                base=-d, channel_multiplier=-1,
                                pattern=[[1, 512]])
        return m

    mask_m64 = affmask("m64", -64)   # di=0 tile: keep c >= p - 64
    mask_d0 = affmask("d0", 0)       # narrowed diag tiles: keep c' >= p

    # ------- weights: one contiguous DMA per [128, 2048] block ----------
    wq_sb = [wpool.tile([128, NKD * 128], BF16, tag=f"wq{m}", name=f"wq{m}")
             for m in range(HPC)]
    wk_sb = wpool.tile([128, NKD * 128], BF16, tag="wk")
    wv_sb = wpool.tile([128, NKD * 128], BF16, tag="wv")
    nc.scalar.dma_start(out=wq_sb[0][:, :], in_=wq_c[0, :, :])
    nc.sync.dma_start(out=wq_sb[1][:, :], in_=wq_c[1, :, :])
    nc.gpsimd.dma_start(out=wk_sb[:, :], in_=wk_c[:, :])
    nc.gpsimd.dma_start(out=wv_sb[:, :], in_=wv_c[:, :])
    wo_sb = None  # loaded after chunk 0

    # ---------------- big persistent activations ----------------
    # KT: col TD+s = seq key s (cols 0:TD = depth keys)
    KT = [big.tile([HD, TD + T], BF16, tag=f"KT{b}", name=f"KT{b}")
          for b in range(B)]
    # VC_sh: shifted V tiles; tile t rows = keys [128t-64, 128t+64);
    # tile 0 rows 0:64 = depth V.
    VC = [big.tile([128, 17 * 128], BF16, tag=f"VC{b}", name=f"VC{b}")
          for b in range(B)]
    VTs = [big.tile([HD, T], BF16, tag=f"VTs{b}", name=f"VTs{b}")
           for b in range(B)]
    for b in range(B):
        nc.sync.dma_start(out=KT[b][:, 0:TD], in_=dkT_c[b, :, :])
        nc.sync.dma_start(out=VC[b][0:TD, 0:128], in_=dv_c[b, :, :])

    Qcur = [None, None]
    xt_cur = [None]

    mask32 = []
    for j in range(16):
        mask32 += [2 * j + 1, 2 * j]

    def load_x(c):
        xt = xp.tile([128, NKD, 512], BF16, tag="xt", name="xt")
        nc.gpsimd.dma_start(out=xt[:, 0:4, :], in_=x_bf[c, :, 0:4 * 512])
        nc.gpsimd.dma_start(out=xt[:, 4:NKD, :], in_=x_bf[c, :, 4 * 512:])
        return xt

    def rsqrt_scaled(ss_ps, out_ri, ln_bias):
        """out_ri = exp(-0.5*ln(ss+eps) + ln_bias) = scale/sqrt(ss)."""
        lnss = rp.tile([128, 512], F32, tag="lnss", name="lnss")
        nc.scalar.activation(lnss[:, :], ss_ps[:, :], AF.Ln,
                             bias=eps_sb[:, :])
        nc.scalar.activation(out_ri, lnss[:, :], AF.Exp, scale=-0.5,
                             bias=ln_bias)

    def rope(qb, n, out_tag):
        """qb: bf16 SBUF copy of the projection (all ops SBUF/bf16)."""
        cs = cos_sb[:, 512 * n:512 * (n + 1)]
        sn = sin_sb[:, 512 * n:512 * (n + 1)]
        swp = rp.tile([128, 512], BF16, tag="swp", name="swp")
        nc.vector.stream_shuffle(swp[:, :], qb[:, :], mask32)
        m1 = rp.tile([128, 512], BF16, tag="m1", name="m1")
        nc.vector.tensor_mul(m1[:, :], qb[:, :], cs)
        m2 = rp.tile([128, 512], BF16, tag="m2", name="m2")
        nc.vector.tensor_mul(m2[:, :], swp[:, :], sn)
        qr = rp.tile([128, 512], BF16, tag=out_tag, name=out_tag)
        nc.vector.tensor_add(qr[:, :], m1[:, :], m2[:, :])
        return qr

    # =========================================================
    def proj_chunk(b, n):
        xt = xt_cur[0]
        pend = []       # (q2_tile, ss_psum, ones_lhsT) queued one behind
        raw = []        # ('q'|'k', h, qr, ss): ss flushed, Ln pending
        lnq = []        # ('q'|'k', h, qr, slot): Ln done into lnc slot
        lnc = rp.tile([128, 3 * 512], F32, tag="lnc", name="lnc")

        def flush_pend():
            while pend:
                q2t, ss, oz = pend.pop(0)
                nc.tensor.matmul(ss[:, :], oz[:, :], q2t[:, :],
                                 start=True, stop=True)

        def ln_ready():
            # Ln for chains whose ss matmuls are flushed, written into
            # slots of ONE tile: the single wide Exp below reads the
            # whole tile, forcing Ln/Ln/Ln/Exp table batching.
            while raw:
                kind, h, qr, ss = raw.pop(0)
                slot = len(lnq)
                nc.scalar.activation(lnc[:, 512 * slot:512 * (slot + 1)],
                                     ss[:, :], AF.Ln, bias=eps_sb[:, :])
                lnq.append((kind, h, qr, slot))

        def chain(w_ap, nm):
            ps = pp.tile([128, 512], F32, tag="psq", name=nm)
            for kk in range(NKD):
                nc.tensor.matmul(ps[:, :], w_ap[:, 128 * kk:128 * (kk + 1)],
                                 xt[:, kk, :],
                                 start=(kk == 0), stop=(kk == NKD - 1))
            flush_pend()
            ln_ready()
            return ps

        for h in range(HPC):
            ps = chain(wq_sb[h], "psq")
            qb = rp.tile([128, 512], BF16, tag="qb", name="qb")
            nc.vector.tensor_copy(qb[:, :], ps[:, :])
            qr = rope(qb, n, "qr")
            q2t = rp.tile([128, 512], BF16, tag="q2h", name="q2h")
            nc.vector.tensor_mul(q2t[:, :], qb[:, :], qb[:, :])
            ss = pt.tile([128, 512], F32, tag="ss", name="ss")
            pend.append((q2t, ss, ones_q[h]))
            raw.append(("q", h, qr, ss))

        ps_k = chain(wk_sb, "psk")
        kb = rp.tile([128, 512], BF16, tag="qb", name="kb")
        nc.vector.tensor_copy(kb[:, :], ps_k[:, :])
        qr_k = rope(kb, n, "qrk")
        q2k = rp.tile([128, 512], BF16, tag="q2h", name="q2k")
        nc.vector.tensor_mul(q2k[:, :], kb[:, :], kb[:, :])
        ss_k = pt.tile([128, 512], F32, tag="ss", name="ssk")
        pend.append((q2k, ss_k, ones_k))
        raw.append(("k", 0, qr_k, ss_k))

        ps_v = chain(wv_sb, "psv")
        ln_ready()

        # ONE wide Exp for all three rsqrt chains, then the norm muls
        ri_all = rp.tile([128, 3 * 512], BF16, tag="riall", name="ri_all")
        nc.scalar.activation(ri_all[:, :], lnc[:, :], AF.Exp, scale=-0.5)
        for kind, h, qr, slot in lnq:
            ri = ri_all[:, 512 * slot:512 * (slot + 1)]
            if kind == "q":
                Qcur[h] = rp.tile([HD, 512], BF16, tag=f"Q{h}",
                                  name=f"Q{h}")
                nc.vector.tensor_mul(Qcur[h][:, :], qr[:, :], ri)
            else:
                nc.vector.tensor_mul(
                    KT[b][:, TD + 512 * n:TD + 512 * (n + 1)],
                    qr[:, :], ri)
        lnq.clear()

        # v = proj + bias -> VTs (v^T), then PE-transpose into VC (shifted)
        vbt_sb = vbp.tile([128, 512], BF16, tag="vbts", name="vbt_sb")
        nc.gpsimd.dma_start(out=vbt_sb[:, :], in_=vb_ch[NCH * b + n, :, :])
        nc.vector.tensor_add(VTs[b][:, 512 * n:512 * (n + 1)],
                             ps_v[:, :], vbt_sb[:, :])
        for j in range(4 * n, 4 * n + 4):
            tp = pl.tile([128, 128], BF16, tag="L", name="tp",
                         padded_shape=[128, 1024])
            nc.tensor.transpose(tp[:, 0:128],
                                VTs[b][:, 128 * j:128 * (j + 1)],
                                ident[:, :])
            # aligned token block j rows 0:64 -> shifted tile j rows 64:128
            nc.vector.tensor_copy(VC[b][64:128, 128 * j:128 * (j + 1)],
                                  tp[0:64, 0:128])
            # rows 64:128 -> shifted tile j+1 rows 0:64
            nc.vector.tensor_copy(VC[b][0:64, 128 * (j + 1):128 * (j + 2)],
                                  tp[64:128, 0:128])

        c_next = NCH * b + n + 1
        if c_next < B * NCH:
            xt_cur[0] = load_x(c_next)

    # =========================================================
    def attn_group(b, g, qlo=0, qhi=512, ym=None):
        # Reference causal mask is top-left aligned on the CONCATENATED
        # [depth | seq] axis: query c attends concat position j <= c,
        # i.e. depth key j <= c and seq key s <= c - TD. On the shifted
        # tile grid (tile kt = concat positions [128kt, 128kt+128), with
        # partition p = concat pos 128kt + p) this is uniformly
        # "keep c >= p + 128*di": no mask below the diagonal, mask_d0 on
        # diagonal tiles, and narrowed slices above. [qlo, qhi) selects a
        # query sub-range of the group (for splitting the tail group).
        c = NCH * b + g
        if ym is None:
            ym = y_mine[c]
        wa = qhi - qlo
        vTg = VTs[b][:, 512 * g + qlo:512 * g + qhi]
        v2g = ap_sb.tile([128, 512], BF16, tag="v2", name="v2")
        nc.gpsimd.tensor_mul(v2g[:, 0:wa], vTg, vTg)
        rv = ap_sb.tile([128, 512], F32, tag="rv", name="rv")

        # tile descriptors: (lhsT_k, lhsT_v, q0, l0, width, mask, npart)
        # q0/l0 = group-/subrange-relative first valid query column.
        tiles = []
        for kt in range(4 * g + 4):
            di = kt - 4 * g
            lo_valid = 128 * di
            if lo_valid >= qhi:
                continue
            q0 = max(qlo, lo_valid)
            mk = None
            if lo_valid + 127 >= q0:
                mk = mask_d0[:, q0 - lo_valid:qhi - lo_valid]
            tiles.append((KT[b][:, 128 * kt:128 * (kt + 1)],
                          VC[b][:, 128 * kt:128 * (kt + 1)],
                          q0, q0 - qlo, qhi - q0, mk, 128))
        ntile = len(tiles)

        for h in range(HPC):
            q_sl = Qcur[h]
            y_ps = py.tile([128, 512], F32, tag="y", name="y_ps")
            z_ps = pz.tile([128, 512], F32, tag="z", name="z_ps")
            Ps = [None] * ntile
            zst = [False]      # z accumulation started
            qsum = [None, 0]   # running quad P-sum (full tiles), count

            def z_emit(rhs_ap, npart, l0, last):
                nc.tensor.matmul(z_ps[:, l0:wa], ones_bf[0:npart, :],
                                 rhs_ap, start=(not zst[0]), stop=last)
                zst[0] = True

            def z_flush():
                if qsum[1] > 0:
                    z_emit(qsum[0], 128, 0, False)
                    qsum[1] = 0

            def accum_y(i):
                _, vt, q0, l0, w, _, npart = tiles[i]
                nc.tensor.matmul(y_ps[:, l0:wa], vt, Ps[i],
                                 start=(i == 0), stop=(i == ntile - 1))

            for i, (kt_sl, vt, q0, l0, w, mk, npart) in enumerate(tiles):
                L = pl.tile([128, 512], F32, tag="L", name="L")
                nc.tensor.matmul(L[0:npart, 0:w], kt_sl,
                                 q_sl[:, q0:qhi], start=True, stop=True)
                P = ap_sb.tile([128, 512], BF16, tag="P", bufs=6, name="P")
                nc.scalar.activation(P[0:npart, 0:w], L[0:npart, 0:w],
                                     AF.Exp, scale=SCALE)
                if mk is not None:
                    nc.vector.tensor_mul(P[0:npart, 0:w], P[0:npart, 0:w],
                                         mk[0:npart, :])
                Ps[i] = P[0:npart, 0:w]
                if mk is None:
                    # full tile: fold ALL fulls' z into one matmul via
                    # cheap bf16 DVE adds
                    if qsum[1] == 0:
                        qsum[0] = Ps[i]
                    else:
                        t = ap_sb.tile([128, 512], BF16, tag="Pq", bufs=3,
                                       name="Pq")
                        nc.vector.tensor_add(t[:, 0:w], qsum[0], Ps[i])
                        qsum[0] = t[:, 0:w]
                    qsum[1] += 1
                else:
                    z_flush()
                    z_emit(Ps[i], npart, l0, i == ntile - 1)
                if i >= 1:
                    accum_y(i - 1)
            accum_y(ntile - 1)

            if h == 0:
                vns = pt.tile([128, 512], F32, tag="ss", name="vns")
                nc.tensor.matmul(vns[:, 0:wa], ones_bf[:, :], v2g[:, 0:wa],
                                 start=True, stop=True)
                nc.vector.reciprocal_approx_fast(out=rv[:, 0:wa],
                                                 in_=vns[:, 0:wa])

            rz = ap_sb.tile([128, 512], F32, tag="rz", name="rz")
            nc.vector.reciprocal_approx_fast(out=rz[:, 0:wa],
                                             in_=z_ps[:, 0:wa])
            yn = ap_sb.tile([128, 512], BF16, tag="yn", name="yn")
            nc.vector.tensor_mul(yn[:, 0:wa], y_ps[:, 0:wa], rz[:, 0:wa])
            yv = ap_sb.tile([128, 512], BF16, tag="yv", name="yv")
            nc.vector.tensor_mul(yv[:, 0:wa], yn[:, 0:wa], vTg)
            dot = pt.tile([128, 512], F32, tag="ss", name="dot")
            nc.tensor.matmul(dot[:, 0:wa], ones_bf[:, :], yv[:, 0:wa],
                             start=True, stop=True)
            coef = ap_sb.tile([128, 512], BF16, tag="coef", name="coef")
            nc.vector.tensor_mul(coef[:, 0:wa], dot[:, 0:wa], rv[:, 0:wa])
            t1 = ap_sb.tile([128, 512], BF16, tag="t1", name="t1")
            nc.vector.tensor_mul(t1[:, 0:wa], coef[:, 0:wa], vTg)
            yf = ap_sb.tile([128, 512], BF16, tag="yf", name="yf")
            nc.vector.tensor_sub(yf[:, 0:wa], yn[:, 0:wa], t1[:, 0:wa])
            nc.gpsimd.dma_start(out=ym[:, h, :], in_=yf[:, 0:wa])
            if KDBG and b == 0 and h == 0 and qlo == 0 and qhi == 512:
                nc.sync.dma_start(out=dbg[:, 512 * g:512 * (g + 1)],
                                  in_=yf[:, :])

    def emit_ag(c):
        nc.gpsimd.collective_compute(
            "AllGather", ALU.bypass, replica_groups=[list(range(NCORES))],
            ins=[y_mine[c][:, :, :]], outs=[y_all[c][:, :, :, :]])

    def emit_ag7(s):
        nc.gpsimd.collective_compute(
            "AllGather", ALU.bypass, replica_groups=[list(range(NCORES))],
            ins=[y_m7[s][:, :, :]], outs=[y_a7[s][:, :, :, :]])

    def load_rhs(c):
        yr = wop.tile([128, NCORES, HPC, 512], BF16, tag="yr", name="yr")
        for r in range(NCORES):
            nc.sync.dma_start(out=yr[:, r, :, :],
                              in_=y_all[c][r, :, :, :])
        return yr

    def emit_wo(c, yr):
        po = [pp.tile([128, 512], F32, tag="psq", name=f"po{m}")
              for m in range(HPC)]
        for cc in range(NKD):
            rhs = yr[:, cc // 2, cc % 2, :]
            for m in range(HPC):
                nc.tensor.matmul(po[m][:, :],
                                 wo_sb[m][:, 128 * cc:128 * (cc + 1)],
                                 rhs, start=(cc == 0), stop=(cc == NKD - 1))
        if KDBG and c == 0:
            nc.sync.dma_start(out=dbg[:, 2048:2560], in_=yr[:, 1, 0, :])
            nc.sync.dma_start(out=dbg[:, 2560:3072], in_=yr[:, 3, 0, :])
            nc.sync.dma_start(out=dbg[:, 3072:3584], in_=yr[:, 6, 1, :])
        if KDBG and c == 1:
            nc.sync.dma_start(out=dbg[:, 4608:5120], in_=yr[:, 0, 0, :])
        for m in range(HPC):
            ob = wos.tile([128, 512], BF16, tag="ob", name="ob")
            nc.scalar.copy(ob[:, :], po[m][:, :])
            if KDBG and c == 0:
                nc.gpsimd.dma_start(out=dbg[:, 3584 + 512 * m:4096 + 512 * m],
                                    in_=ob[:, :])
            nc.gpsimd.dma_start(
                out=outT[128 * m:128 * (m + 1), 512 * c:512 * (c + 1)],
                in_=ob[:, :])

    # =========================================================
    xt_cur[0] = load_x(0)
    rhs_t = {}
    for b in range(B):
        for n in range(NCH):
            c = NCH * b + n
            proj_chunk(b, n)
            attn_group(b, n)
            emit_ag(c)
            if c == 0:
                wo_sb = [wpool.tile([128, NKD * 128], BF16, tag=f"wo{m}",
                                    name=f"wo{m}") for m in range(HPC)]
                nc.scalar.dma_start(out=wo_sb[0][:, :], in_=wo_c[0, :, :])
                nc.sync.dma_start(out=wo_sb[1][:, :], in_=wo_c[1, :, :])
            if c >= 1:
                rhs_t[c - 1] = load_rhs(c - 1)
            if c >= 2:
                emit_wo(c - 2, rhs_t.pop(c - 2))
    rhs_t[7] = load_rhs(7)
    emit_wo(6, rhs_t.pop(6))
    emit_wo(7, rhs_t.pop(7))

    for p in (pz, py, pt, pl, pp, wos, wop, ap_sb, vbp, rp, xp,
              big, wpool, const):
        p.release()


_NC_CACHE = None


def _get_nc():
    global _NC_CACHE
    if _NC_CACHE is None:
        _NC_CACHE = _build()
    return _NC_CACHE


def _pack_w(w_rows, nblk):
    """w_rows: [nblk*128 out-rows, DIM] f32 -> [nblk, 128, NKD*128] bf16
    packed so lhsT tile (m, kk) = buf[m][:, 128kk:128kk+128]."""
    import ml_dtypes
    s = w_rows.reshape(nblk, 128, NKD, 128)        # [m, col, kk, p]
    s = s.transpose(0, 3, 2, 1)                    # [m, p, kk, col]
    return np.ascontiguousarray(
        s.reshape(nblk, 128, NKD * 128).astype(ml_dtypes.bfloat16))


def _shard_inputs(inputs):
    import ml_dtypes
    BF = ml_dtypes.bfloat16
    x = np.asarray(inputs["x"], np.float32)
    fc = np.asarray(inputs["freqs_cos"], np.float32)
    fs = np.asarray(inputs["freqs_sin"], np.float32)
    vb = np.asarray(inputs["value_bias"], np.float32)
    dk = np.asarray(inputs["depth_k"], np.float32)
    dv = np.asarray(inputs["depth_v"], np.float32)
    wq = np.asarray(inputs["wq"], np.float32)
    wk = np.asarray(inputs["wk"], np.float32)
    wv = np.asarray(inputs["wv"], np.float32)
    wo = np.asarray(inputs["wo"], np.float32)
    qs = np.asarray(inputs["q_scale"], np.float32).reshape(H)
    ks = np.asarray(inputs["k_scale"], np.float32).reshape(KVH)

    xT = x.reshape(TOK, DIM).T                     # [DIM, TOK]
    # x_bf[c, p, kk*512+t] = xT[128kk+p, 512c+t]
    x_bf = np.ascontiguousarray(
        xT.reshape(NKD, 128, B * NCH, 512).transpose(2, 1, 0, 3)
        .reshape(B * NCH, 128, NKD * 512).astype(BF))
    cosT = np.ascontiguousarray(np.repeat(fc.T, 2, axis=0).astype(BF))
    sinT = np.repeat(fs.T, 2, axis=0).copy()
    sinT[0::2] *= -1.0
    sinT = np.ascontiguousarray(sinT.astype(BF))
    vbf = vb.reshape(TOK, KVH * HD)

    maps = []
    for c in range(NCORES):
        kvh = c // 2
        vbT = vbf[:, HD * kvh:HD * (kvh + 1)].T    # [HD, TOK]
        vb_c = np.ascontiguousarray(
            vbT.reshape(HD, B * NCH, 512).transpose(1, 0, 2).astype(BF))
        m = {
            "x_bf": x_bf,
            "wq_c": _pack_w(wq[256 * c:256 * (c + 1)], HPC),
            "wk_c": _pack_w(wk[HD * kvh:HD * (kvh + 1)], 1)[0],
            "wv_c": _pack_w(wv[HD * kvh:HD * (kvh + 1)], 1)[0],
            # wo: lhsT[p, col] = wo[256c+128m+col, 128cc+p] -> pack wo rows
            # like wq but with contraction = head-dim (wo columns)
            "wo_c": _pack_w(wo[256 * c:256 * (c + 1)], HPC),
            "vb_ch": vb_c,
            "dkT_c": np.ascontiguousarray(
                dk[:, kvh].transpose(0, 2, 1).astype(BF)),
            "dv_c": np.ascontiguousarray(dv[:, kvh].astype(BF)),
            "cosT": cosT,
            "sinT": sinT,
            "qs_c": np.ascontiguousarray(
                np.broadcast_to(qs[2 * c:2 * c + 2][None, :], (128, 2))),
            "ks_c": np.full((128, 1), ks[kvh], np.float32),
        }
        maps.append(m)
    return maps


def _gather_output(results):
    outT = np.concatenate(
        [np.asarray(results[c]["outT"], dtype=np.float32)
         for c in range(NCORES)], axis=0)
    return np.ascontiguousarray(outT.T).reshape(B, T, DIM).astype(np.float32)


def kernel(**inputs):
    from concourse import bass_utils
    nc = _get_nc()
    from concourse.bass_interp import get_hw_module
    maps = _shard_inputs(inputs)
    old = nc.m
    nc.m = get_hw_module(nc.m)
    try:
        res = bass_utils.run_bass_kernel_spmd(nc, maps, list(range(NCORES)))
    finally:
        nc.m = old
    return _gather_output(res.results)
